# revision 1
# baseline (speedup 1.0000x reference)
"""Causal self-attention on Trainium2, tensor-parallel over heads across 8 NeuronCores.

Strategy (sharding_hint "tensor-parallel split the n_heads axis"):
  - Each core c owns heads {2c, 2c+1} == columns [128c, 128c+128) of Wq/Wk/Wv
    and rows [128c, 128c+128) of Wo.
  - Per core: QT/KT = (x @ W{q,k})^T in [feat, tok] layout, V in [tok, feat]
    layout with an appended ones column (denominator trick).
  - Scores are computed transposed ([k, q] layout) so exp(scoresT) feeds the
    PV matmul directly (lhsT = V_aug stationary, rhs = attnT streaming), which
    also yields the softmax denominators as row 64 of the PV output.
  - Normalization: reciprocal of the denominator row, broadcast across
    partitions with a K=1 matmul, multiply -> attnoutT [feat, tok].
  - Partial out-projection y_c = attnout_c @ Wo_c; host sums the 8 partials
    and adds bo.  (All-reduce done on host: gather/unshard step.)

Matmul inputs are bf16 (PSUM accumulation fp32): single-pass matmuls + FWL
weight loads, vs fp32's LOW_HIGH double pass.
"""

import sys

if "/opt/trn_rl_repo" not in sys.path:
    sys.path.insert(0, "/opt/trn_rl_repo")

from contextlib import ExitStack

import ml_dtypes
import numpy as np

import concourse.bass as bass
import concourse.mybir as mybir
import concourse.tile as tile

F32 = mybir.dt.float32
BF = mybir.dt.bfloat16
NPBF = ml_dtypes.bfloat16
EXP = mybir.ActivationFunctionType.Exp
LN = mybir.ActivationFunctionType.Ln
COPY = mybir.ActivationFunctionType.Copy

P = 128  # partition tile
HD = 64  # head dim
HC = 2  # heads per core (HC*HD == P)
WIN = 512  # token window (one PSUM bank of fp32)
MASK_VAL = -30000.0
N_WARM = 48  # PE warm-up matmuls (run under the x-load DMA shadow)


def _legalize_waits(nc):
    """This walrus build encodes at most ONE semaphore wait per instruction
    (setupSyncWait raises "Too many sync wait commands" otherwise).  Tile
    freely emits 2+ waits, so excess waits are moved onto injected same-engine
    NoOps (one wait each) directly before the instruction."""
    nop_id = 0
    for fn in nc.m.functions:
        for blk in fn.blocks:
            out = []
            for inst in blk.instructions:
                if type(inst).__name__ != "InstNoOp":
                    si = inst.sync_info
                    waits = list(si.on_wait or []) if si is not None else []
                    if len(waits) > 1:
                        for w in waits[1:]:
                            nop = mybir.InstNoOp(
                                name=f"nopw-{nop_id}",
                                engine=inst.engine,
                                ins=[],
                                outs=[],
                                sync_info=mybir.SyncInfo(on_wait=[w], on_update=[]),
                            )
                            nop_id += 1
                            out.append(nop)
                        si.on_wait = waits[:1]
                out.append(inst)
            blk.instructions[:] = out


def build_nc(B, T, D, n_cores, debug_dump=False, legalize=True):
    """Build the SPMD Bass program (same program all cores, per-core data).

    The two batches are software-pipelined: batch b+1's QKV projection
    matmuls (full-array, stall-free) are interleaved into batch b's
    attention stream, and batch b+1's attention interleaves with batch b's
    out-projection."""
    nj = D // P  # contraction tiles for projections
    n_win = T // WIN  # q windows per batch
    n_qt = T // P  # token tiles per batch
    M = B * T
    VW = 2 * P  # V_aug cols per token tile: per head [V(64) | one | zeros(63)]

    nc = bass.Bass("TRN2", target_bir_lowering=False, debug=False, num_devices=n_cores)

    xt = nc.dram_tensor("xt", [D, M], BF, kind="ExternalInput").ap()
    wq = nc.dram_tensor("wq", [P, D], BF, kind="ExternalInput").ap()
    wk = nc.dram_tensor("wk", [P, D], BF, kind="ExternalInput").ap()
    wv = nc.dram_tensor("wv", [P, D], BF, kind="ExternalInput").ap()
    wo = nc.dram_tensor("wo", [P, D], BF, kind="ExternalInput").ap()
    bq = nc.dram_tensor("bq", [1, P], BF, kind="ExternalInput").ap()
    bk = nc.dram_tensor("bk", [1, P], BF, kind="ExternalInput").ap()
    bv = nc.dram_tensor("bv", [1, P], BF, kind="ExternalInput").ap()
    msk = nc.dram_tensor("msk", [P, P], BF, kind="ExternalInput").ap()
    y = nc.dram_tensor("y", [M, D], BF, kind="ExternalOutput").ap()

    with tile.TileContext(nc) as tc, ExitStack() as ctx:
        const = ctx.enter_context(tc.tile_pool(name="const", bufs=1))
        xtp = ctx.enter_context(tc.tile_pool(name="xt", bufs=2 * nj + 2))
        qkp = ctx.enter_context(tc.tile_pool(name="qk", bufs=2))
        vp = ctx.enter_context(tc.tile_pool(name="vaug", bufs=2))
        atp = ctx.enter_context(tc.tile_pool(name="attnT", bufs=8))
        aop = ctx.enter_context(tc.tile_pool(name="aoT", bufs=2))
        pvp = ctx.enter_context(tc.tile_pool(name="pvs", bufs=8))
        rcp = ctx.enter_context(tc.tile_pool(name="rc", bufs=6))
        yp = ctx.enter_context(tc.tile_pool(name="ysb", bufs=6))
        # PSUM budget (8 banks): sc 2x2-bank pairs + pv 2x1 + proj/y 2x1
        ps_sc = ctx.enter_context(tc.tile_pool(name="ps_sc", bufs=2, space="PSUM"))
        ps_pv = ctx.enter_context(tc.tile_pool(name="ps_pv", bufs=2, space="PSUM"))
        ps_proj = ctx.enter_context(tc.tile_pool(name="ps_proj", bufs=2, space="PSUM"))

        # constants / weights
        wq_s = const.tile([P, D], BF, tag="wq")
        wk_s = const.tile([P, D], BF, tag="wk")
        wv_s = const.tile([P, D], BF, tag="wv")
        wo_s = const.tile([P, D], BF, tag="wo")
        bq_s = const.tile([1, P], BF, tag="bq")
        bk_s = const.tile([1, P], BF, tag="bk")
        bv_s = const.tile([1, P], BF, tag="bv")
        msk_s = const.tile([P, P], BF, tag="msk")
        ones_r = const.tile([1, WIN], BF, tag="ones")
        ones_f = const.tile([1, HD], F32, tag="onesf")
        warm_s = const.tile([P, WIN], BF, tag="warm")
        nc.vector.memset(ones_r[:, :], 1.0)
        nc.vector.memset(ones_f[:, :], 1.0)
        nc.vector.memset(warm_s[:, :], 1.0)
        nc.sync.dma_start(wq_s[:, :], wq[:, :])
        nc.sync.dma_start(wk_s[:, :], wk[:, :])
        nc.sync.dma_start(wv_s[:, :], wv[:, :])
        nc.sync.dma_start(wo_s[:, :], wo[:, :])
        nc.sync.dma_start(bq_s[:, :], bq[:, :])
        nc.sync.dma_start(bk_s[:, :], bk[:, :])
        nc.sync.dma_start(bv_s[:, :], bv[:, :])
        nc.sync.dma_start(msk_s[:, :], msk[:, :])

        # PE warm-up: dense dummy matmuls while the first x tiles stream in,
        # so the HAM clock gate reaches 8/8 before the real work starts.
        psw = ps_proj.tile([P, WIN], F32, tag="proj")
        for i in range(N_WARM):
            nc.tensor.matmul(
                psw[:, :], warm_s[:, 0:P], warm_s[:, :], start=True, stop=True
            )

        st = {}  # per-batch pipeline state

        def load_xt(b):
            toff = b * T
            xts = []
            for j in range(nj):
                xt_t = xtp.tile([P, T], BF, tag="xt", name=f"xt{b}_{j}")
                nc.sync.dma_start(xt_t[:, :], xt[j * P : (j + 1) * P, toff : toff + T])
                xts.append(xt_t)
            qts = [
                qkp.tile([P, T], BF, tag=f"qt{h}", name=f"qt{b}_{h}")
                for h in range(HC)
            ]
            kts = [
                qkp.tile([P, T], BF, tag=f"kt{h}", name=f"kt{b}_{h}")
                for h in range(HC)
            ]
            for tl in qts + kts:  # rows HD..P stay zero: full-K=128 scores
                nc.vector.memset(tl[HD:P, :], 0.0)
            st[b] = {
                "xts": xts,
                "qt": qts,
                "kt": kts,
                "vaug": vp.tile([P, n_qt * VW], BF, tag="vaug", name=f"vaug{b}"),
                "aoT": aop.tile([P, T], BF, tag="aoT", name=f"aoT{b}"),
                "pend": [],
                "vready": False,
            }

        def proj_chunk(b, w, which):
            s = st[b]
            ws = w * WIN
            w_s, b_s, dst = (
                (wq_s, bq_s, s["qt"]) if which == "q" else (wk_s, bk_s, s["kt"])
            )
            psp = ps_proj.tile([P, WIN], F32, tag="proj", name=f"ps{which}{b}_{w}")
            for j in range(nj):
                nc.tensor.matmul(
                    psp[:, :],
                    w_s[:, j * P : (j + 1) * P],
                    s["xts"][j][:, ws : ws + WIN],
                    start=(j == 0),
                    stop=False,
                )
            nc.tensor.matmul(
                psp[:, :], b_s[:, :], ones_r[:, :], start=False, stop=True
            )
            for h in range(HC):
                nc.vector.tensor_copy(
                    dst[h][0:HD, ws : ws + WIN], psp[h * HD : (h + 1) * HD, :]
                )

        def v_tile(b, t):
            s = st[b]
            if not s["vready"]:
                va3 = s["vaug"].rearrange("p (t c) -> p t c", c=P)
                nc.vector.memset(va3[:, :, HD : HD + 1], 1.0)  # ones col
                nc.vector.memset(va3[:, :, HD + 1 : P], 0.0)  # zero pad
                s["vready"] = True
            base = t * VW
            psv = ps_proj.tile([P, P], F32, tag="proj", name=f"psv{b}_{t}")
            for j in range(nj):
                nc.tensor.matmul(
                    psv[:, :],
                    s["xts"][j][:, t * P : (t + 1) * P],
                    wv_s[:, j * P : (j + 1) * P],
                    start=(j == 0),
                    stop=False,
                )
            nc.tensor.matmul(
                psv[:, :], ones_r[:, 0:P], bv_s[:, :], start=False, stop=True
            )
            nc.vector.tensor_copy(s["vaug"][:, base : base + HD], psv[:, 0:HD])
            nc.vector.tensor_copy(
                s["vaug"][:, base + P : base + P + HD], psv[:, HD : 2 * HD]
            )

        def qkv_thunks(b, w):
            th = [
                lambda b=b, w=w: proj_chunk(b, w, "q"),
                lambda b=b, w=w: proj_chunk(b, w, "k"),
            ]
            for t in range(w * n_qt // n_win, (w + 1) * n_qt // n_win):
                th.append(lambda b=b, t=t: v_tile(b, t))
            return th

        def qkv_window(b, w):
            for f in qkv_thunks(b, w):
                f()

        def normalize(b, pvsb, rc, hp, ws):
            # aoT[h, w] = pv[0:HD] * recip(denom)-broadcast; traced one window
            # late so the PE never stalls on the DVE reciprocal.
            psb = ps_sc.tile([HD, WIN], F32, tag="sc", name=f"psb{b}_{hp}_{ws}")
            nc.tensor.matmul(psb[:, :], ones_f[:, :], rc[:, :], start=True, stop=True)
            nc.vector.tensor_mul(
                st[b]["aoT"][hp : hp + HD, ws : ws + WIN], pvsb[0:HD, :], psb[:, :]
            )

        def attn_window(b, w, filler=None):
            # Heads interleaved: their K=64 score matmuls sit adjacent in the
            # PE stream, so the disjoint row groups execute concurrently.
            # k tiles two at a time: both score chunks of a head land in one
            # 2-bank PSUM tile, one exp per pair; PV for pair p is traced
            # after the scores of pair p+1 so the PE never waits on the exp.
            s = st[b]
            qt_s, kt_s, vaug, pend = s["qt"], s["kt"], s["vaug"], s["pend"]
            ws = w * WIN
            njt = (ws + WIN) // P  # causal k tiles for this window
            pspv = [
                ps_pv.tile([P, WIN], F32, tag="pv", name=f"pspv{b}_{w}_{_h}")
                for _h in range(HC)
            ]

            def flush_pv(at, halves):
                for h in range(HC):
                    for j, off, N, qstart in halves[h]:
                        vb = j * VW + h * P
                        nc.tensor.matmul(
                            pspv[h][:, qstart - ws : WIN],
                            vaug[:, vb : vb + P],
                            at[h][:, off : off + N],
                            start=(j == 0),
                            stop=(j == njt - 1),
                        )

            prev = None
            for j0 in range(0, njt, 2):
                pss = [
                    ps_sc.tile([P, 2 * WIN], F32, tag="sc", name=f"pss{_h}")
                    for _h in range(HC)
                ]
                if not filler:
                    # no real interleave work left: one full-array dummy
                    # matmul into the score tile (about to be overwritten)
                    # keeps the HAM clock gate registering PE activity.
                    nc.tensor.matmul(
                        pss[0][:, 0:WIN], warm_s[:, 0:P], warm_s[:, :],
                        start=True, stop=True,
                    )
                at = [
                    atp.tile([P, 2 * WIN], BF, tag="at", name=f"at{_h}")
                    for _h in range(HC)
                ]
                halves = [[] for _ in range(HC)]
                off = [0] * HC
                for j in (j0, j0 + 1):
                    if j >= njt:
                        continue
                    qstart = max(ws, j * P)
                    N = ws + WIN - qstart
                    for h in range(HC):
                        hp = h * HD
                        o = off[h]
                        if o and o + N > WIN:
                            o = WIN  # don't straddle a PSUM bank
                        nc.tensor.matmul(
                            pss[h][:, o : o + N],
                            kt_s[h][:, j * P : (j + 1) * P],
                            qt_s[h][:, qstart : qstart + N],
                            start=True,
                            stop=True,
                        )
                        halves[h].append((j, o, N, qstart))
                        off[h] = o + N
                for h in range(HC):
                    width = halves[h][-1][1] + halves[h][-1][2]
                    nc.scalar.activation(at[h][:, 0:width], pss[h][:, 0:width], EXP)
                    for j, o, N, qstart in halves[h]:
                        if j * P >= ws:  # zero the upper triangle post-exp
                            nc.vector.tensor_mul(
                                at[h][:, o : o + P], at[h][:, o : o + P],
                                msk_s[:, :],
                            )
                if prev is not None:
                    flush_pv(*prev)
                    if filler:
                        filler.pop(0)()
                prev = (at, halves)
            flush_pv(*prev)
            if filler:
                filler.pop(0)()
            for h in range(HC):
                pvsb = pvp.tile([HD + 1, WIN], F32, tag="pvs", name=f"pvsb{h}")
                nc.vector.tensor_copy(pvsb[:, :], pspv[h][0 : HD + 1, :])
                lg = rcp.tile([1, WIN], F32, tag="lg", name=f"lg{h}")
                nc.scalar.activation(lg[:, :], pspv[h][HD : HD + 1, :], LN)
                rc = rcp.tile([1, WIN], F32, tag="rc", name=f"rc{h}")
                nc.scalar.activation(rc[:, :], lg[:, :], EXP, scale=-1.0)
                pend.append((pvsb, rc, h * HD, ws))
                while len(pend) > 2:
                    normalize(b, *pend.pop(0))

        def flush_norms(b):
            for args in st[b]["pend"]:
                normalize(b, *args)
            st[b]["pend"] = []

        def outproj_tile(b, t):
            toff = b * T
            aoT = st[b]["aoT"]
            ysb = yp.tile([P, D], BF, tag="ysb", name=f"ysb{b}_{t}")
            for ui, u0 in enumerate(range(0, D, WIN)):
                N = min(WIN, D - u0)
                psy = ps_proj.tile([P, WIN], F32, tag="proj", name=f"psy{b}_{t}_{ui}")
                nc.tensor.matmul(
                    psy[:, 0:N],
                    aoT[:, t * P : (t + 1) * P],
                    wo_s[:, u0 : u0 + N],
                    start=True,
                    stop=True,
                )
                nc.vector.tensor_copy(ysb[:, u0 : u0 + N], psy[:, 0:N])
            nc.sync.dma_start(y[toff + t * P : toff + (t + 1) * P, :], ysb[:, :])

        # ---- pipelined schedule: QKV(b+1) / out-proj(b-1) thunks are
        # interleaved between attention pair-groups as uniform PE filler ----
        load_xt(0)
        for w in range(n_win):
            qkv_window(0, w)
        for b in range(B):
            filler = []
            if b + 1 < B:
                load_xt(b + 1)
                for w in range(n_win):
                    filler.extend(qkv_thunks(b + 1, w))
            if b > 0:
                filler.extend(
                    (lambda bb=b - 1, t=t: outproj_tile(bb, t)) for t in range(n_qt)
                )
            tpw = n_qt // n_win
            done_t = set()
            for w in range(n_win):
                attn_window(b, w, filler)
                if b == B - 1 and w > 0:
                    # last batch: own out-proj tiles as filler (2-window lag;
                    # their normalizes were traced during this window's evacs)
                    for t in range((w - 1) * tpw, w * tpw):
                        filler.append(lambda bb=b, t=t: outproj_tile(bb, t))
                        done_t.add(t)
            for f in filler:
                f()
            flush_norms(b)
            if b > 0:
                del st[b - 1]
        for t in range(n_qt):
            if t not in done_t:
                outproj_tile(B - 1, t)
        del st[B - 1]

    if legalize:
        _legalize_waits(nc)
    return nc


def make_in_maps(x, Wq, bq, Wk, bk, Wv, bv, Wo, n_cores):
    x = np.asarray(x, dtype=np.float32)
    Bb, Tt, Dd = x.shape
    M = Bb * Tt
    xt = np.ascontiguousarray(x.reshape(M, Dd).T.astype(NPBF))
    mask = np.where(
        np.arange(P)[:, None] > np.arange(P)[None, :], 0.0, 1.0
    ).astype(NPBF)

    def wslice(W, c, scale=1.0):
        Wc = np.asarray(W, np.float32)[:, c * P : (c + 1) * P] * np.float32(scale)
        return np.ascontiguousarray(
            Wc.reshape(Dd // P, P, P).transpose(1, 0, 2).reshape(P, Dd).astype(NPBF)
        )

    qscale = 1.0 / np.sqrt(HD)
    in_maps = []
    for c in range(n_cores):
        cs = slice(c * P, (c + 1) * P)
        in_maps.append(
            {
                "xt": xt,
                "wq": wslice(Wq, c, qscale),
                "wk": wslice(Wk, c),
                "wv": wslice(Wv, c),
                "wo": np.ascontiguousarray(
                    np.asarray(Wo, np.float32)[cs, :].astype(NPBF)
                ),
                "bq": np.ascontiguousarray(
                    (np.asarray(bq, np.float32)[cs] * np.float32(qscale))
                    .reshape(1, P)
                    .astype(NPBF)
                ),
                "bk": np.ascontiguousarray(
                    np.asarray(bk, np.float32)[cs].reshape(1, P).astype(NPBF)
                ),
                "bv": np.ascontiguousarray(
                    np.asarray(bv, np.float32)[cs].reshape(1, P).astype(NPBF)
                ),
                "msk": mask,
            }
        )
    return in_maps


_NC_CACHE = {}


def get_nc(B, T, D, n_cores):
    key = (B, T, D, n_cores)
    if key not in _NC_CACHE:
        _NC_CACHE[key] = build_nc(B, T, D, n_cores)
    return _NC_CACHE[key]


def kernel(**inputs):
    from concourse.bass_utils import run_bass_kernel_spmd

    x = np.asarray(inputs["x"], np.float32)
    Bb, Tt, Dd = x.shape
    n_cores = 8
    nc = get_nc(Bb, Tt, Dd, n_cores)
    in_maps = make_in_maps(
        x,
        inputs["Wq"],
        inputs["bq"],
        inputs["Wk"],
        inputs["bk"],
        inputs["Wv"],
        inputs["bv"],
        inputs["Wo"],
        n_cores,
    )
    res = run_bass_kernel_spmd(nc, in_maps, core_ids=list(range(n_cores)))
    y = np.zeros((Bb * Tt, Dd), dtype=np.float64)
    for r in res.results:
        y += r["y"].astype(np.float64)
    y += np.asarray(inputs["bo"], np.float64)[None, :]
    return y.reshape(Bb, Tt, Dd).astype(np.float32)

